# revision 29
# baseline (speedup 1.0000x reference)
"""Trainium2 Bass kernel for nn_Block_73615739454081 (tri-block sparse attention + FFN).

Contract: kernel(**inputs) takes FULL unsharded inputs (as produced by
setup_inputs()) and returns the FULL [1, N, D] float32 output.

Strategy (8 NeuronCores, SPMD):
  - Shard the block axis: 161 blocks of 256 tokens. Each core runs an identical
    program over 21 "local" blocks plus a 1-block halo on each side (23-block
    slab). Adjacent cores overlap by one block; the gather keeps a disjoint
    20/20/.../21 split. No collectives: the halo is materialized host-side.
  - Host precomputes LN1 (it depends only on the input x) and uploads the
    normalized activations feature-major (hn1T, bf16), so the device never
    transposes in the attention phase. Conditioning scale/offset and the
    1/sqrt(d) factor are folded into the weights host-side.
  - Attention runs in transposed layouts: scores ST = [keys, queries] so the
    exp'd scores feed the AV matmul directly; softmax denominators come from
    ones-column matmuls accumulated in PSUM, inverted with the fast DVE
    reciprocal, and broadcast back with a K=1 matmul. Softmax tails are
    software-pipelined one head behind the matmul stream so the PE never
    stalls on them.
  - Phase A (attention, exp-only on ACT) and phase B (FFN, gelu-only) are
    fully separated so the ACT engine loads each activation table once.
    LN2 statistics are computed during phase A (bn_stats + a Newton-iteration
    rsqrt on the vector engine), so phase B needs no sqrt table either.
  - Everything flows in bf16 matmul operands (fp32 PSUM accumulation).
"""

import numpy as np

# ---------------------------------------------------------------- constants
N = 40962
D = 512
H = 4
HD = 128
BS = 256
NB = 161
NP = NB * BS
C = 16
FFW = 4 * D
SCALE = HD ** -0.5
EPS = 1e-5

NCORES = 8
LOCAL = 21                 # local blocks per core (uniform SPMD program)
SLAB = LOCAL + 2           # +1 halo block each side
SNODES = SLAB * BS         # 5888 slab tokens
KT = SNODES // 128         # 46 key tiles of 128 in the slab
OUT_NODES = LOCAL * BS     # 5376
NT2 = OUT_NODES // 128     # 42 output row tiles of 128
STARTS = [0, 20, 40, 60, 80, 100, 120, 140]   # first local block per core
# kt tiles that can contain invalid keys on some core (slab edges):
FIXUP_KTS = (0, 1, KT - 4, KT - 3, KT - 2, KT - 1)

# groups over the slab: g0=[j0], g1..g10=[j(2m-1), j(2m)], g11=[j21, j22]
# chain m (m=1..11): queries = local blocks of group m, keys = 1-block halo.
NGROUPS = 12

_PROG_CACHE = {}


# ---------------------------------------------------------------- device code
def _build_program(has_bf, has_bd, debug_stage=None):
    import concourse.bass as bass  # noqa: F401
    import concourse.mybir as mybir
    import concourse.tile as tile
    from concourse import bacc

    F32 = mybir.dt.float32
    F32R = mybir.dt.float32r
    BF16 = mybir.dt.bfloat16
    I32 = mybir.dt.int32
    AF = mybir.ActivationFunctionType
    OP = mybir.AluOpType

    nc = bacc.Bacc("TRN2", target_bir_lowering=False, debug=False)

    def din(name, shape, dt):
        return nc.dram_tensor(name, shape, dt, kind="ExternalInput").ap()

    hn1T_d = din("hn1T", [128, 4 * SNODES], BF16)   # feature-major LN1(x)
    xloc_d = din("xloc", [OUT_NODES, D], F32)       # token-major local x
    km_d = din("kmask", [128, KT], F32)
    wqT_d = din("wqT", [128, 4 * H * HD], BF16)
    wkT_d = din("wkT", [128, 4 * H * HD], BF16)
    wvN_d = din("wvN", [128, 4 * D], BF16)
    wfN_d = din("wfN", [128, H * D], BF16)
    wupT_d = din("wupT", [128, 4 * FFW], BF16)
    wdnN_d = din("wdnN", [128, 16 * D], BF16)
    cqB_d = din("cqB", [128, H], F32)
    ckB_d = din("ckB", [128, H], F32)
    cvB_d = din("cvB", [128, D], F32)
    cuB_d = din("cuB", [128, 16], F32)
    identB_d = din("identB", [128, 128], BF16)
    onesC_d = din("onesC", [128, 1], BF16)
    magic_d = din("magicB", [128, 1], I32)
    if has_bf:
        bfB_d = din("bfB", [128, D], F32)
    if has_bd:
        bdB_d = din("bdB", [128, D], F32)
    out_d = nc.dram_tensor("out", [OUT_NODES, D], F32, kind="ExternalOutput").ap()

    # group geometry: group g covers slab blocks blist, query cols qw
    def gblocks(g):
        if g == 0:
            return [0]
        return [2 * g - 1, 2 * g] if 2 * g < SLAB else [2 * g - 1]

    with nc.allow_low_precision(reason="bf16 matmul operands / outputs by design"), \
         tile.TileContext(nc) as tc:
        with (
            tc.tile_pool(name="wconst", bufs=1) as wpool,
            tc.tile_pool(name="sb", bufs=2) as sb,
            tc.tile_pool(name="ps", bufs=2, space="PSUM") as ps,
        ):
            # ---------------- resident weights / constants
            def wtile(name, dram, shape, dt):
                t = wpool.tile(shape, dt, name=name)
                nc.sync.dma_start(out=t, in_=dram)
                return t

            # weight tiles are DMA'd inside the emission section (after the
            # priming hn1T DMAs) so phase A isn't delayed; names are assigned
            # there and resolved by the closures below at call time.
            wqT = wkT = wvN = wfN = cqB = ckB = cvB = None
            onesC = magicB = kmt = bfB = None
            wupT = wdnN = cuB = identB = bdB = None

            # resident state across phases
            r1_all = wpool.tile([128, NT2 * D], BF16, name="r1_all")
            mvs = wpool.tile([128, 2 * NT2], F32, name="mvs")
            rstds = wpool.tile([128, NT2], F32, name="rstds")

            hn1T = {}   # (g, c) -> [128, 512] bf16
            qT2 = {}    # (g, h) -> [128, qw] bf16
            kT2 = {}    # (g, h) -> [128, kw] bf16
            vtl = {}    # (g, tt) -> [128, 512] bf16

            # ---------------- phase 1 (q/k/v projection) as filler units
            def phase1_units(g):
                """Return list of closures: DMAs+matmul groups for group g."""
                blist = gblocks(g)
                ncol = len(blist) * BS          # token columns in this group
                c0 = blist[0] * BS              # first token in slab
                units = []

                def dma_unit():
                    for c in range(4):
                        t = sb.tile([128, ncol], BF16, tag="hn1T", bufs=12,
                                    name=f"h1T{g}_{c}")
                        nc.sync.dma_start(
                            out=t, in_=hn1T_d[:, c * SNODES + c0:
                                              c * SNODES + c0 + ncol])
                        hn1T[(g, c)] = t
                units.append(dma_unit)

                has_q = [b for b in blist if 1 <= b <= LOCAL]

                # proj/v emitted as single-matmul micro-steps so chain-side
                # short matmuls can hide their LDWEIGHTS under filler streams.
                # Only fillers allocate the p1 ring and units run in order, so
                # interleaving micro-steps of one unit with chain matmuls
                # (st/ot/dn tags) is safe.
                def proj_steps(which, h):
                    st = {}

                    def step(c):
                        wT, cB = (wqT, cqB) if which == "q" else (wkT, ckB)
                        if which == "q":
                            qc0 = (has_q[0] - blist[0]) * BS
                            ncq = len(has_q) * BS
                        else:
                            qc0, ncq = 0, ncol
                        if c == 0:
                            st["p"] = ps.tile([128, 512], F32, tag="p1", bufs=2,
                                              name=f"p{which}{g}_{h}")
                        pQ = st["p"]
                        nc.tensor.matmul(
                            pQ[:, 0:ncq],
                            lhsT=wT[:, (c * H + h) * 128:(c * H + h) * 128 + 128],
                            rhs=hn1T[(g, c)][:, qc0:qc0 + ncq],
                            start=(c == 0), stop=(c == 3),
                        )
                        if c == 3:
                            outt = sb.tile([128, ncq], BF16, tag=f"{which}T2",
                                           bufs=(8 if which == "q" else 12),
                                           name=f"{which}T{g}_{h}")
                            nc.vector.tensor_scalar(
                                outt, pQ[:, 0:ncq], cB[:, h:h + 1], None, OP.add)
                            if which == "q":
                                qT2[(g, h)] = outt
                            else:
                                kT2[(g, h)] = outt
                    return [lambda c=c: step(c) for c in range(4)]

                for h in range(H):
                    units.extend(proj_steps("k", h))
                if has_q:
                    for h in range(H):
                        units.extend(proj_steps("q", h))

                def v_steps(tt):
                    st = {}

                    def step(c):
                        if c == 0:
                            st["p"] = ps.tile([128, 512], F32, tag="p1", bufs=2,
                                              name=f"pV{g}_{tt}")
                        pV = st["p"]
                        nc.tensor.matmul(
                            pV,
                            lhsT=hn1T[(g, c)][:, tt * 128:tt * 128 + 128],
                            rhs=wvN[:, c * D:c * D + D],
                            start=(c == 0), stop=(c == 3),
                        )
                        if c == 3:
                            vt = sb.tile([128, D], BF16, tag="vring", bufs=14,
                                         name=f"v{g}_{tt}")
                            nc.vector.tensor_tensor(vt, pV, cvB, OP.add)
                            vtl[(g, tt)] = vt
                    return [lambda c=c: step(c) for c in range(4)]

                for tt in range(2 * len(blist)):
                    units.extend(v_steps(tt))
                return units

            # ---------------- newton rsqrt for LN2 (vector engine only)
            def ln2_stats(src, t):
                stats = sb.tile([128, 6], F32, tag="stats", bufs=4, name=f"st{t}")
                nc.vector.bn_stats(out=stats, in_=src)
                nc.vector.bn_aggr(out=mvs[:, 2 * t:2 * t + 2], in_=stats)
                veps = sb.tile([128, 1], F32, tag="nw0", bufs=3, name=f"ve{t}")
                nc.vector.tensor_scalar(
                    veps, mvs[:, 2 * t + 1:2 * t + 2], EPS, None, OP.add)
                sh = sb.tile([128, 1], F32, tag="nw1", bufs=3, name=f"sh{t}")
                nc.vector.tensor_scalar(
                    sh.bitcast(I32), veps.bitcast(I32), 1, None,
                    OP.arith_shift_right)
                y0 = sb.tile([128, 1], F32, tag="nw2", bufs=3, name=f"y0{t}")
                nc.vector.tensor_tensor(
                    y0.bitcast(I32), magicB, sh.bitcast(I32), OP.subtract)
                # two NR iterations: rstd = y*(1.5 - 0.5*veps*y^2)
                cur = y0
                for it in range(2):
                    a = sb.tile([128, 1], F32, tag=f"nw{3 + 2 * it}", bufs=3,
                                name=f"a{t}_{it}")
                    nc.vector.tensor_tensor(a, cur, cur, OP.mult)
                    nc.vector.tensor_tensor(a, a, veps, OP.mult)
                    nc.vector.tensor_scalar(a, a, -0.5, 1.5, OP.mult, OP.add)
                    nxt = (sb.tile([128, 1], F32, tag=f"nw{4 + 2 * it}", bufs=3,
                                   name=f"y{t}_{it}")
                           if it == 0 else rstds[:, t:t + 1])
                    nc.vector.tensor_tensor(nxt, cur, a, OP.mult)
                    cur = nxt

            # ---------------- attention chain for group m (local pair)
            def chain(m, fillers):
                qlist = [b for b in gblocks(m) if 1 <= b <= LOCAL]
                qw = len(qlist) * BS
                b0 = 2 * (m - 1)                 # first local block index
                kt_lo = 2 * (qlist[0] - 1)
                kt_hi = 2 * (qlist[-1] + 1) + 2  # exclusive
                kts = list(range(kt_lo, kt_hi))

                def vbis(kt):
                    jk = kt // 2
                    return [bi for bi, j in enumerate(qlist) if abs(jk - j) <= 1]

                mid = [kt for kt in kts if len(vbis(kt)) == len(qlist)]
                rest = [kt for kt in kts if len(vbis(kt)) != len(qlist)]
                order = mid + rest

                def gof(kt):
                    j = kt // 2
                    if j == 0:
                        return (0, kt)
                    g = (j + 1) // 2
                    base = 2 * gblocks(g)[0]
                    return (g, kt - base)

                def crange(kt):
                    vb = vbis(kt)
                    return vb[0] * BS, (vb[-1] + 1) * BS

                # pace filler micro-steps evenly across all chain matmuls
                nsteps = 3 * H * len(order)
                nfill0 = len(fillers)
                fstate = {"step": 0, "done": 0}

                def fill():
                    fstate["step"] += 1
                    want = (fstate["step"] * nfill0) // nsteps
                    while fstate["done"] < want and fillers:
                        fillers.pop(0)()
                        fstate["done"] += 1

                oo = {}
                tails = []
                for h in range(H):
                    oT = ps.tile([128, qw], F32, tag="ot", bufs=2, name=f"oT{m}_{h}")
                    dn = ps.tile([1, qw], F32, tag="dn", bufs=1, name=f"dn{m}_{h}")
                    ets = {}
                    # software pipeline: scores lead exp/dn/oT by 2 kts
                    for idx, kt in enumerate(order + [None, None]):
                        if kt is not None:
                            c0, c1 = crange(kt)
                            STp = ps.tile([128, qw], F32, tag="st", bufs=3,
                                          name=f"S{m}_{h}_{kt}")
                            g, off = gof(kt)
                            nc.tensor.matmul(
                                STp[:, c0:c1],
                                lhsT=kT2[(g, h)][:, off * 128:off * 128 + 128],
                                rhs=qT2[(m, h)][:, c0:c1],
                                start=True, stop=True,
                            )
                            ets[kt] = STp
                            fill()
                        if idx >= 2:
                            kt2 = order[idx - 2]
                            c0, c1 = crange(kt2)
                            STp2 = ets.pop(kt2)
                            Et = sb.tile([128, qw], BF16, tag="E", bufs=4,
                                         name=f"E{m}_{h}_{kt2}")
                            nc.scalar.activation(Et[:, c0:c1], STp2[:, c0:c1],
                                                 AF.Exp)
                            if kt2 in FIXUP_KTS:
                                nc.vector.tensor_scalar(
                                    Et[:, c0:c1], Et[:, c0:c1],
                                    kmt[:, kt2:kt2 + 1], None, OP.mult)
                            first = (idx - 2) == 0
                            last = (idx - 2) == len(order) - 1
                            nc.tensor.matmul(dn[:, c0:c1], lhsT=onesC,
                                             rhs=Et[:, c0:c1],
                                             start=first, stop=last)
                            fill()
                            g2, off2 = gof(kt2)
                            nc.tensor.matmul(
                                oT[:, c0:c1],
                                lhsT=vtl[(g2, off2)][:, h * HD:h * HD + HD],
                                rhs=Et[:, c0:c1],
                                start=first, stop=last)
                            fill()

                    # reciprocal eagerly (frees the single dn PSUM bank before
                    # the next head's accumulation); rest of the tail delayed
                    rdn = sb.tile([1, qw], F32, tag="rdn", bufs=4,
                                  name=f"rd{m}_{h}")
                    nc.vector.reciprocal_approx_fast(out=rdn, in_=dn)

                    def tail(h=h, oT=oT, rdn=rdn):
                        rdnBs = sb.tile([128, qw], F32, tag="rdnBs", bufs=2,
                                        name=f"rBs{m}_{h}")
                        nc.gpsimd.partition_broadcast(rdnBs, rdn, channels=128)
                        ooh = sb.tile([128, qw], BF16, tag="oo", bufs=8,
                                      name=f"oo{m}_{h}")
                        nc.vector.tensor_tensor(ooh, oT, rdnBs, OP.mult)
                        oo[h] = ooh
                    tails.append(tail)
                    # run previous head's tail now (pipelined by one head)
                    if len(tails) >= 2:
                        tails.pop(0)()
                tails.pop(0)()

                # final projection + residual + LN2 stats
                for qs in range(qw // 128):
                    at = ps.tile([128, D], F32, tag="st", bufs=3,
                                 name=f"at{m}_{qs}")
                    for h in range(H):
                        nc.tensor.matmul(
                            at,
                            lhsT=oo[h][:, qs * 128:qs * 128 + 128],
                            rhs=wfN[:, h * D:h * D + D],
                            start=(h == 0), stop=(h == 3),
                        )
                    xres = sb.tile([128, D], F32, tag="xres", bufs=3,
                                   name=f"xr{m}_{qs}")
                    r0 = b0 * BS + qs * 128
                    nc.sync.dma_start(out=xres, in_=xloc_d[r0:r0 + 128, :])
                    t = b0 * 2 + qs
                    rt = r1_all[:, t * D:(t + 1) * D]
                    if has_bf:
                        rtf = sb.tile([128, D], F32, tag="rtf", bufs=2,
                                      name=f"rtf{m}_{qs}")
                        nc.vector.tensor_tensor(rtf, at, xres, OP.add)
                        nc.vector.tensor_tensor(rt, rtf, bfB, OP.add)
                    else:
                        nc.vector.tensor_tensor(rt, at, xres, OP.add)
                    if debug_stage == "r1":
                        ot32 = sb.tile([128, D], F32, tag="dbg", bufs=2,
                                       name=f"dbg{m}_{qs}")
                        nc.vector.tensor_copy(ot32, rt)
                        nc.sync.dma_start(
                            out=out_d[t * 128:t * 128 + 128, :], in_=ot32)
                    ln2_stats(rt, t)

            # ---------------- phase B: FFN for one 512-token chunk
            def ffn_chunk(u):
                """u indexes 512-token chunks of the 42 output row tiles."""
                ntile = min(4, NT2 - 4 * u)
                qw = ntile * 128
                # LN2 apply (token-major) -> hn2 bf16
                hn2 = []
                for i in range(ntile):
                    t = 4 * u + i
                    ht = sb.tile([128, D], BF16, tag="hn2", bufs=5, name=f"hn2_{t}")
                    nc.vector.tensor_scalar(
                        ht, r1_all[:, t * D:(t + 1) * D],
                        mvs[:, 2 * t:2 * t + 1], rstds[:, t:t + 1],
                        OP.subtract, OP.mult)
                    hn2.append(ht)
                # transposes -> h2T bf16 [128, qw] per d-chunk
                h2T = []
                for c in range(4):
                    hps = ps.tile([128, 512], BF16, tag="st", bufs=3,
                                  name=f"hp{u}_{c}")
                    for i in range(ntile):
                        nc.tensor.transpose(
                            hps[:, i * 128:i * 128 + 128],
                            hn2[i][:, c * 128:c * 128 + 128],
                            identB,
                        )
                    ht = sb.tile([128, qw], BF16, tag="h2T", bufs=5,
                                 name=f"h2{u}_{c}")
                    nc.vector.tensor_copy(ht, hps[:, 0:qw])
                    h2T.append(ht)
                # FFN up + gelu
                gl = []
                for fb in range(16):
                    g = ps.tile([128, qw], F32, tag="ot", bufs=2, name=f"g{u}_{fb}")
                    for c in range(4):
                        nc.tensor.matmul(
                            g,
                            lhsT=wupT[:, (c * 16 + fb) * 128:(c * 16 + fb) * 128 + 128],
                            rhs=h2T[c],
                            start=(c == 0), stop=(c == 3),
                        )
                    gt = sb.tile([128, qw], BF16, tag="gl", bufs=17,
                                 name=f"gl{u}_{fb}")
                    nc.scalar.activation(gt, g, AF.Gelu_apprx_tanh,
                                         bias=cuB[:, fb:fb + 1])
                    gl.append(gt)
                # FFN down + residual + store
                for i in range(ntile):
                    t = 4 * u + i
                    y = ps.tile([128, D], F32, tag="p1", bufs=2, name=f"y{u}_{i}")
                    for fb in range(16):
                        nc.tensor.matmul(
                            y,
                            lhsT=gl[fb][:, i * 128:i * 128 + 128],
                            rhs=wdnN[:, fb * D:fb * D + D],
                            start=(fb == 0), stop=(fb == 15),
                        )
                    ot = sb.tile([128, D], F32, tag="outt", bufs=3,
                                 name=f"ot{u}_{i}")
                    nc.vector.tensor_tensor(
                        ot, y, r1_all[:, t * D:(t + 1) * D], OP.add)
                    if has_bd:
                        nc.vector.tensor_tensor(ot, ot, bdB, OP.add)
                    nc.sync.dma_start(
                        out=out_d[t * 128:t * 128 + 128, :], in_=ot)

            # ---------------- emission
            # phase A: prime groups 0..2, then chain m with group m+2 as filler
            f0 = phase1_units(0)
            f1 = phase1_units(1)
            f2 = phase1_units(2)
            # hn1T priming DMAs first, then phase-A weights, then the rest
            f0[0]()
            f1[0]()
            f2[0]()
            wkT = wtile("wkT", wkT_d, [128, 4 * H * HD], BF16)
            wqT = wtile("wqT", wqT_d, [128, 4 * H * HD], BF16)
            ckB = wtile("ckB", ckB_d, [128, H], F32)
            cqB = wtile("cqB", cqB_d, [128, H], F32)
            wvN = wtile("wvN", wvN_d, [128, 4 * D], BF16)
            cvB = wtile("cvB", cvB_d, [128, D], F32)
            wfN = wtile("wfN", wfN_d, [128, H * D], BF16)
            onesC = wtile("onesC", onesC_d, [128, 1], BF16)
            magicB = wtile("magicB", magic_d, [128, 1], I32)
            kmt = wtile("kmt", km_d, [128, KT], F32)
            if has_bf:
                bfB = wtile("bfB", bfB_d, [128, D], F32)
            for u in f0[1:] + f1[1:] + f2[1:]:
                u()
            # phase-B weights: DMA'd after priming so they don't delay phase A
            wupT = wtile("wupT", wupT_d, [128, 4 * FFW], BF16)
            wdnN = wtile("wdnN", wdnN_d, [128, 16 * D], BF16)
            cuB = wtile("cuB", cuB_d, [128, 16], F32)
            identB = wtile("identB", identB_d, [128, 128], BF16)
            if has_bd:
                bdB = wtile("bdB", bdB_d, [128, D], F32)
            for m in range(1, 12):
                fillers = phase1_units(m + 2) if m + 2 < NGROUPS else []
                chain(m, fillers)
                for u in fillers:
                    u()
            if debug_stage == "r1":
                pass
            else:
                # phase B
                nchunks = (NT2 + 3) // 4
                for u in range(nchunks):
                    ffn_chunk(u)

    nc.compile()
    return nc


# ---------------------------------------------------------------- host side
def _prep(inputs):
    import ml_dtypes
    f8 = np.float64
    x = np.asarray(inputs["x"], np.float32).reshape(N, D)
    mask = np.asarray(inputs["mask"])
    gnc = np.asarray(inputs["global_norm_conditioning"], np.float32)
    wq = np.asarray(inputs["wq"], np.float32)
    wk = np.asarray(inputs["wk"], np.float32)
    wv = np.asarray(inputs["wv"], np.float32)
    w_final = np.asarray(inputs["w_final"], np.float32)
    b_final = np.asarray(inputs["b_final"], np.float32)
    w_up = np.asarray(inputs["w_up"], np.float32)
    b_up = np.asarray(inputs["b_up"], np.float32)
    w_down = np.asarray(inputs["w_down"], np.float32)
    b_down = np.asarray(inputs["b_down"], np.float32)
    w_cond = np.asarray(inputs["w_cond"], np.float32)
    b_cond = np.asarray(inputs["b_cond"], np.float32)

    so = gnc.astype(f8) @ w_cond.astype(f8) + b_cond.astype(f8)
    sc = 1.0 + so[0, :D]
    off = so[0, D:]

    wq2 = wq.astype(f8) * sc[:, None] * SCALE
    cq = (off @ wq.astype(f8)) * SCALE
    wk2 = wk.astype(f8) * sc[:, None]
    ck = off @ wk.astype(f8)
    wv2 = wv.astype(f8) * sc[:, None]
    cv = off @ wv.astype(f8)
    wu2 = w_up.astype(f8) * sc[:, None]
    cu = off @ w_up.astype(f8) + b_up.astype(f8)

    def tobf(a):
        return np.ascontiguousarray(a, np.float32).astype(ml_dtypes.bfloat16)

    def to32(a):
        return np.ascontiguousarray(a, np.float32)

    dev = {}
    dev["wqT"] = tobf(wq2.reshape(4, 128, H, HD).transpose(1, 0, 2, 3).reshape(128, -1))
    dev["wkT"] = tobf(wk2.reshape(4, 128, H, HD).transpose(1, 0, 2, 3).reshape(128, -1))
    dev["wvN"] = tobf(wv2.reshape(4, 128, D).transpose(1, 0, 2).reshape(128, -1))
    dev["wfN"] = tobf(
        w_final.astype(f8).reshape(H, HD, D).transpose(1, 0, 2).reshape(HD, -1))
    dev["wupT"] = tobf(
        wu2.reshape(4, 128, 16, 128).transpose(1, 0, 2, 3).reshape(128, -1))
    dev["wdnN"] = tobf(
        w_down.astype(f8).reshape(16, 128, D).transpose(1, 0, 2).reshape(128, -1))
    dev["cqB"] = to32(cq.reshape(H, HD).T)
    dev["ckB"] = to32(ck.reshape(H, HD).T)
    dev["cvB"] = to32(np.tile(cv[None, :], (128, 1)))
    dev["cuB"] = to32(cu.reshape(16, 128).T)
    dev["identB"] = np.eye(128, dtype=np.float32).astype(ml_dtypes.bfloat16)
    dev["onesC"] = np.ones((128, 1), np.float32).astype(ml_dtypes.bfloat16)
    dev["magicB"] = np.full((128, 1), 0x5f3759df, np.int32)

    has_bf = bool(np.any(b_final != 0))
    has_bd = bool(np.any(b_down != 0))
    if has_bf:
        dev["bfB"] = to32(np.tile(b_final[None, :], (128, 1)))
    if has_bd:
        dev["bdB"] = to32(np.tile(b_down[None, :], (128, 1)))

    # host LN1 over padded x
    xpad = np.zeros((NP, D), np.float32)
    xpad[:N] = x
    mu1 = xpad.mean(axis=1, keepdims=True)
    var1 = xpad.var(axis=1, keepdims=True)
    hn1 = (xpad - mu1) * (1.0 / np.sqrt(var1 + EPS))

    # global key validity from the diagonal mask (keys of block n)
    kv_global = np.asarray(mask[0, 0, :, 0, 0, :], bool).reshape(NP)

    per_core = []
    for c in range(NCORES):
        g0 = (STARTS[c] - 1) * BS
        hs = np.zeros((SNODES, D), np.float32)
        lo = max(0, -g0)
        hi = min(SNODES, NP - g0)
        hs[lo:hi] = hn1[g0 + lo:g0 + hi]
        # feature-major: [128, 4*SNODES], chunk-major layout
        hn1T = np.ascontiguousarray(
            hs.T.reshape(4, 128, SNODES).transpose(1, 0, 2).reshape(128, -1)
        ).astype(ml_dtypes.bfloat16)
        xl = np.ascontiguousarray(
            xpad[STARTS[c] * BS: STARTS[c] * BS + OUT_NODES])
        kmv = np.zeros(SNODES, bool)
        kmv[lo:hi] = kv_global[g0 + lo:g0 + hi]
        km_t = kmv.reshape(KT, 128)
        for kt in range(KT):
            if not km_t[kt].all():
                assert kt in FIXUP_KTS, f"unexpected invalid keys at kt={kt}"
        per_core.append({
            "hn1T": hn1T,
            "xloc": xl,
            "kmask": np.ascontiguousarray(km_t.T.astype(np.float32)),
        })
    return dev, per_core, has_bf, has_bd


def _run(inputs, trace=False, trace_kwargs=None):
    from concourse.bass_utils import run_bass_kernel_spmd

    import os
    dbg = os.environ.get("KERNEL_DEBUG_STAGE") or None
    dev, per_core, has_bf, has_bd = _prep(inputs)
    key = (has_bf, has_bd, dbg)
    if key not in _PROG_CACHE:
        _PROG_CACHE[key] = _build_program(has_bf, has_bd, debug_stage=dbg)
    nc = _PROG_CACHE[key]

    in_maps = []
    for c in range(NCORES):
        m = dict(dev)
        m.update(per_core[c])
        in_maps.append(m)
    kw = {}
    if trace:
        kw["trace"] = True
        if trace_kwargs:
            kw.update(trace_kwargs)
    res = run_bass_kernel_spmd(nc, in_maps, list(range(NCORES)), **kw)

    out = np.zeros((NP, D), np.float32)
    for c in range(NCORES):
        nblk = NB - STARTS[c] if c == NCORES - 1 else STARTS[c + 1] - STARTS[c]
        rows = nblk * BS
        out[STARTS[c] * BS: STARTS[c] * BS + rows] = res.results[c]["out"][:rows]
    x_in = np.asarray(inputs["x"])
    return out[:N].reshape(1, N, D).astype(x_in.dtype), res


def kernel(**inputs):
    out, _ = _run(inputs)
    return out
